# revision 1
# baseline (speedup 1.0000x reference)
"""Trainium2 Bass kernel for the Logic-Model (temporal point process) log-likelihood.

Reference math (S=4096 samples, H=3 heads, E=512 events, G=3334 grid pts, F=1):
    w_eff[h] = weights[h,0] * effects[h,0]
    ev_logit[s,h,e] = bases[h] + w_eff[h] * event_features[s,h,e,0]
    gr_logit[s,h,g] = bases[h] + w_eff[h] * grid_features[s,h,g,0]
    out = sum(mask * ev_logit) - 0.03 * sum(exp(gr_logit))

Decomposition used here (exact algebra, per head h):
    sum(mask * ev_logit) = bases[h] * count_h + w_eff[h] * sum(mask * ev)
    sum(exp(gr_logit))   = exp(bases[h]) * sum(exp(w_eff[h] * gr))

Device work per core (data-parallel over samples, 512 samples/core):
    - exp-sums of w_eff[h]*grid via ScalarE activation(Exp, scale) with fused
      row-accumulate (accum_out), one call per (tile, head)
    - masked event sums + mask counts via VectorE mul + segmented reduce
    - emits a [128, 36] partials tensor (per-partition sums); host combines in
      float64 (the "all-reduce" of the scalar log-likelihood).

HBM traffic per core = 20.5MB grid + 3.1MB events + 0.8MB mask ~= 24.4MB
-> ~68us at the ~358GB/s per-NeuronCore HBM limit; compute (ACT ~37us,
DVE ~24us) hides under the DMA.
"""

import numpy as np

S, H, E, G = 4096, 3, 512, 3334
N_CORES = 8
S_LOCAL = S // N_CORES   # 512 samples per core
P = 128                  # SBUF partitions
N_TILES = S_LOCAL // P   # 4 tiles of 128 samples
INTEGRAL_RESOLUTION = 0.03

# partials column layout: [exp sums | masked-ev sums | mask counts], each
# N_TILES*H columns indexed t*H+h
N_COLS = 3 * N_TILES * H

_build_cache = {}


def _build(w_eff):
    """Build the per-core Bass program. w_eff: tuple of H python floats baked
    in as activation scale immediates (kernel is re-built per weight set)."""
    import concourse.bacc as bacc
    import concourse.mybir as mybir
    from concourse.tile import TileContext

    f32 = mybir.dt.float32

    nc = bacc.Bacc(trn_type="TRN2", target_bir_lowering=False, debug=False)

    ev = nc.dram_tensor("ev", [S_LOCAL, H, E], f32, kind="ExternalInput")
    mk = nc.dram_tensor("mk", [S_LOCAL, H, E], mybir.dt.uint8, kind="ExternalInput")
    gr = nc.dram_tensor("gr", [S_LOCAL, H, G], f32, kind="ExternalInput")
    partials = nc.dram_tensor("partials", [P, N_COLS], f32, kind="ExternalOutput")

    with TileContext(nc) as tc, \
            tc.tile_pool(name="grp", bufs=2) as grp, \
            tc.tile_pool(name="evp", bufs=2) as evp, \
            tc.tile_pool(name="mkp", bufs=2) as mkp, \
            tc.tile_pool(name="scr", bufs=2) as scr, \
            tc.tile_pool(name="accp", bufs=1) as accp:
        acc = accp.tile([P, N_COLS], f32)

        for t in range(N_TILES):
            r0, r1 = t * P, (t + 1) * P

            gr_t = grp.tile([P, H, G], f32)
            nc.sync.dma_start(out=gr_t[:], in_=gr[r0:r1])
            ev_t = evp.tile([P, H, E], f32)
            nc.sync.dma_start(out=ev_t[:], in_=ev[r0:r1])
            mk_t = mkp.tile([P, H, E], mybir.dt.uint8)
            nc.sync.dma_start(out=mk_t[:], in_=mk[r0:r1])

            # grid: exp(w_h * g) with fused per-row sum into acc column t*H+h
            for h in range(H):
                c = t * H + h
                nc.scalar.activation(
                    out=gr_t[:, h, :],
                    in_=gr_t[:, h, :],
                    func=mybir.ActivationFunctionType.Exp,
                    scale=float(w_eff[h]),
                    accum_out=acc[:, c:c + 1],
                )

            # events: cast mask u8->f32, count + masked sum, segmented by head
            mkf = scr.tile([P, H, E], f32, tag="mkf")
            nc.vector.tensor_copy(mkf[:], mk_t[:])
            nc.vector.reduce_sum(
                out=acc[:, 2 * N_TILES * H + t * H: 2 * N_TILES * H + (t + 1) * H],
                in_=mkf[:],
                axis=mybir.AxisListType.X,
            )
            prod = scr.tile([P, H, E], f32, tag="prod")
            nc.vector.tensor_mul(prod[:], ev_t[:], mkf[:])
            nc.vector.reduce_sum(
                out=acc[:, N_TILES * H + t * H: N_TILES * H + (t + 1) * H],
                in_=prod[:],
                axis=mybir.AxisListType.X,
            )

        nc.sync.dma_start(out=partials[:, :], in_=acc[:])

    nc.compile()
    return nc


def _run_on_device(in_maps, w_eff, trace=False):
    from concourse.bass_utils import run_bass_kernel_spmd

    key = tuple(round(float(x), 12) for x in w_eff)
    if key not in _build_cache:
        _build_cache.clear()
        _build_cache[key] = _build(w_eff)
    nc = _build_cache[key]
    return run_bass_kernel_spmd(
        nc, in_maps, core_ids=list(range(N_CORES)), trace=trace
    )


def _prep_in_maps(inputs):
    ev = np.ascontiguousarray(
        np.asarray(inputs["event_features"], dtype=np.float32).reshape(S, H, E))
    mk = np.asarray(inputs["event_mask"]).reshape(S, H, E).view(np.uint8)
    gr = np.ascontiguousarray(
        np.asarray(inputs["grid_features"], dtype=np.float32).reshape(S, H, G))
    return [
        {
            "ev": ev[c * S_LOCAL:(c + 1) * S_LOCAL],
            "mk": mk[c * S_LOCAL:(c + 1) * S_LOCAL],
            "gr": gr[c * S_LOCAL:(c + 1) * S_LOCAL],
        }
        for c in range(N_CORES)
    ]


def _combine(partials_list, w_eff, bases):
    """Host-side all-reduce + final scalar combine, in float64."""
    sums = np.zeros(N_COLS, dtype=np.float64)
    for part in partials_list:
        sums += part.astype(np.float64).sum(axis=0)
    n = N_TILES * H
    exp_s = sums[0:n].reshape(N_TILES, H).sum(axis=0)          # [H]
    mev_s = sums[n:2 * n].reshape(N_TILES, H).sum(axis=0)      # [H]
    cnt_s = sums[2 * n:3 * n].reshape(N_TILES, H).sum(axis=0)  # [H]

    b = np.asarray(bases, dtype=np.float64)
    w = np.asarray(w_eff, dtype=np.float64)
    log_sum = float(np.sum(b * cnt_s + w * mev_s))
    integral = INTEGRAL_RESOLUTION * float(np.sum(np.exp(b) * exp_s))
    return np.float32(log_sum - integral)


def kernel(**inputs):
    w_eff = (np.asarray(inputs["weights"], dtype=np.float32)[:, 0]
             * np.asarray(inputs["effects"], dtype=np.float32)[:, 0])
    bases = np.asarray(inputs["bases"], dtype=np.float32)

    in_maps = _prep_in_maps(inputs)
    res = _run_on_device(in_maps, w_eff)
    partials_list = [r["partials"] for r in res.results]
    return _combine(partials_list, w_eff, bases)


# revision 2
# speedup vs baseline: 1.0891x; 1.0891x over previous
"""Trainium2 Bass kernel for the Logic-Model (temporal point process) log-likelihood.

Reference math (S=4096 samples, H=3 heads, E=512 events, G=3334 grid pts, F=1):
    w_eff[h] = weights[h,0] * effects[h,0]
    ev_logit[s,h,e] = bases[h] + w_eff[h] * event_features[s,h,e,0]
    gr_logit[s,h,g] = bases[h] + w_eff[h] * grid_features[s,h,g,0]
    out = sum(mask * ev_logit) - 0.03 * sum(exp(gr_logit))

Decomposition used here (exact algebra, per head h):
    sum(mask * ev_logit) = bases[h] * count_h + w_eff[h] * sum(mask * ev)
    sum(exp(gr_logit))   = exp(bases[h]) * sum(exp(w_eff[h] * gr))

Device work per core (data-parallel over samples, 512 samples/core):
    - exp-sums of w_eff[h]*grid via ScalarE activation(Exp, scale) with fused
      row-accumulate (accum_out), one call per (tile, head)
    - masked event sums + mask counts via VectorE mul + segmented reduce
    - emits a [128, 36] partials tensor (per-partition sums); host combines in
      float64 (the "all-reduce" of the scalar log-likelihood).

HBM traffic per core = 20.5MB grid + 3.1MB events + 0.8MB mask ~= 24.4MB
-> ~68us at the ~358GB/s per-NeuronCore HBM limit; compute (ACT ~37us,
DVE ~24us) hides under the DMA.
"""

import numpy as np

S, H, E, G = 4096, 3, 512, 3334
N_CORES = 8
S_LOCAL = S // N_CORES   # 512 samples per core
P = 128                  # SBUF partitions
N_TILES = S_LOCAL // P   # 4 tiles of 128 samples
INTEGRAL_RESOLUTION = 0.03

# partials column layout: [exp sums | masked-ev sums | mask counts], each
# N_TILES*H columns indexed t*H+h
N_COLS = 3 * N_TILES * H

_build_cache = {}


def _build(w_eff, repeat=1):
    """Build the per-core Bass program. w_eff: tuple of H python floats baked
    in as activation scale immediates (kernel is re-built per weight set).
    repeat > 1 re-runs the whole body (same data) for benchmarking."""
    import concourse.bacc as bacc
    import concourse.mybir as mybir
    from concourse.tile import TileContext

    f32 = mybir.dt.float32

    nc = bacc.Bacc(trn_type="TRN2", target_bir_lowering=False, debug=False)

    ev = nc.dram_tensor("ev", [S_LOCAL, H, E], f32, kind="ExternalInput")
    mk = nc.dram_tensor("mk", [S_LOCAL, H, E], mybir.dt.uint8, kind="ExternalInput")
    gr = nc.dram_tensor("gr", [S_LOCAL, H, G], f32, kind="ExternalInput")
    partials = nc.dram_tensor("partials", [P, N_COLS], f32, kind="ExternalOutput")

    with TileContext(nc) as tc, \
            tc.tile_pool(name="grp", bufs=2) as grp, \
            tc.tile_pool(name="evp", bufs=2) as evp, \
            tc.tile_pool(name="mkp", bufs=2) as mkp, \
            tc.tile_pool(name="scr", bufs=2) as scr, \
            tc.tile_pool(name="accp", bufs=1) as accp:
        acc = accp.tile([P, N_COLS], f32)

        for t in [t for _ in range(repeat) for t in range(N_TILES)]:
            r0, r1 = t * P, (t + 1) * P

            gr_t = grp.tile([P, H, G], f32)
            nc.sync.dma_start(out=gr_t[:], in_=gr[r0:r1])
            ev_t = evp.tile([P, H, E], f32)
            nc.sync.dma_start(out=ev_t[:], in_=ev[r0:r1])
            mk_t = mkp.tile([P, H, E], mybir.dt.uint8)
            nc.sync.dma_start(out=mk_t[:], in_=mk[r0:r1])

            # grid: exp(w_h * g) with fused per-row sum into acc column t*H+h
            for h in range(H):
                c = t * H + h
                nc.scalar.activation(
                    out=gr_t[:, h, :],
                    in_=gr_t[:, h, :],
                    func=mybir.ActivationFunctionType.Exp,
                    scale=float(w_eff[h]),
                    accum_out=acc[:, c:c + 1],
                )

            # events: cast mask u8->f32, count + masked sum, segmented by head
            mkf = scr.tile([P, H, E], f32, tag="mkf")
            nc.vector.tensor_copy(mkf[:], mk_t[:])
            nc.vector.reduce_sum(
                out=acc[:, 2 * N_TILES * H + t * H: 2 * N_TILES * H + (t + 1) * H],
                in_=mkf[:],
                axis=mybir.AxisListType.X,
            )
            prod = scr.tile([P, H, E], f32, tag="prod")
            nc.vector.tensor_mul(prod[:], ev_t[:], mkf[:])
            nc.vector.reduce_sum(
                out=acc[:, N_TILES * H + t * H: N_TILES * H + (t + 1) * H],
                in_=prod[:],
                axis=mybir.AxisListType.X,
            )

        nc.sync.dma_start(out=partials[:, :], in_=acc[:])

    nc.compile()
    return nc


def _run_on_device(in_maps, w_eff, trace=False):
    from concourse.bass_utils import run_bass_kernel_spmd

    key = tuple(round(float(x), 12) for x in w_eff)
    if key not in _build_cache:
        _build_cache.clear()
        _build_cache[key] = _build(w_eff)
    nc = _build_cache[key]
    return run_bass_kernel_spmd(
        nc, in_maps, core_ids=list(range(N_CORES)), trace=trace
    )


def _prep_in_maps(inputs):
    ev = np.ascontiguousarray(
        np.asarray(inputs["event_features"], dtype=np.float32).reshape(S, H, E))
    mk = np.asarray(inputs["event_mask"]).reshape(S, H, E).view(np.uint8)
    gr = np.ascontiguousarray(
        np.asarray(inputs["grid_features"], dtype=np.float32).reshape(S, H, G))
    return [
        {
            "ev": ev[c * S_LOCAL:(c + 1) * S_LOCAL],
            "mk": mk[c * S_LOCAL:(c + 1) * S_LOCAL],
            "gr": gr[c * S_LOCAL:(c + 1) * S_LOCAL],
        }
        for c in range(N_CORES)
    ]


def _combine(partials_list, w_eff, bases):
    """Host-side all-reduce + final scalar combine, in float64."""
    sums = np.zeros(N_COLS, dtype=np.float64)
    for part in partials_list:
        sums += part.astype(np.float64).sum(axis=0)
    n = N_TILES * H
    exp_s = sums[0:n].reshape(N_TILES, H).sum(axis=0)          # [H]
    mev_s = sums[n:2 * n].reshape(N_TILES, H).sum(axis=0)      # [H]
    cnt_s = sums[2 * n:3 * n].reshape(N_TILES, H).sum(axis=0)  # [H]

    b = np.asarray(bases, dtype=np.float64)
    w = np.asarray(w_eff, dtype=np.float64)
    log_sum = float(np.sum(b * cnt_s + w * mev_s))
    integral = INTEGRAL_RESOLUTION * float(np.sum(np.exp(b) * exp_s))
    return np.float32(log_sum - integral)


def kernel(**inputs):
    w_eff = (np.asarray(inputs["weights"], dtype=np.float32)[:, 0]
             * np.asarray(inputs["effects"], dtype=np.float32)[:, 0])
    bases = np.asarray(inputs["bases"], dtype=np.float32)

    in_maps = _prep_in_maps(inputs)
    res = _run_on_device(in_maps, w_eff)
    partials_list = [r["partials"] for r in res.results]
    return _combine(partials_list, w_eff, bases)
